# revision 5
# baseline (speedup 1.0000x reference)
"""GCN node classifier on 8 Trainium2 NeuronCores — dma_gather edition.

3-layer GCN, N=100000 nodes, E=3.2M edges, d_in=512, d_h=32, n_cls=40.

Math refactor (linearity of gcn_conv), same as the previous kernel:
    segsum(norm * (h@W)[src]) + b  ==  (dinv * segsum((dinv * h)[src])) @ W + b
so per-edge `norm` folds into node-level dinv scaling and the self-loop
becomes a node-local term (hloc).

Sharding: nodes dst-sharded 12500/core.  Per layer each core gathers the
(dinv-scaled) features of every in-edge source from a replicated table and
segment-sums them into its 98 dst windows with one-hot matmuls on TensorE.

What changed vs the indirect-DMA version: the per-edge gather now uses
InstDMAGatherAnt (1024 indices per instruction instead of 128, ~8x less
fixed overhead), which requires 256B table rows (features padded 32->128
bf16 cols) and int16 indices.  int16 forces a 4-way split of the node space;
the split is chosen as shard-slice j of every core so the table quarter T_j
is exactly the output of an AllGather of shard rows [j*NL/4, (j+1)*NL/4).
The 4 per-slice collectives let next-layer gathers start after the first
quarter lands.  Rows are 1 edge each; padding rows carry segrel=-1 so their
one-hot row is all zero and the gathered garbage never contributes.
"""

import math
import numpy as np
import ml_dtypes

BF16 = ml_dtypes.bfloat16

# ---------------------------------------------------------------- config

class Cfg:
    def __init__(self, n_nodes, n_edges, d_in=512, d_h=32, n_cls=40, n_cores=8,
                 cw=4):
        assert n_nodes % n_cores == 0
        self.N = n_nodes
        self.E = n_edges
        self.C = n_cores
        self.NL = n_nodes // n_cores          # nodes per core
        assert self.NL % 4 == 0
        self.SL = self.NL // 4                # shard-slice rows (int16 space /C)
        self.TJ = n_cores * self.SL           # rows of one table quarter
        assert self.TJ <= 32767
        self.WN = 112                         # nodes per dst window (<128 so a
                                              # (window, slice) fits one gather)
        self.TN = math.ceil(self.NL / self.WN)
        self.NLP = self.TN * 128              # window-slab rows of padded x
        self.D_IN = d_in
        self.DH = d_h
        self.DC = n_cls
        self.EPS = 1e-5
        self.CW = cw                          # windows per gather/scatter chunk
        self.GMAX = 1024                      # dma_gather index cap

FULL = Cfg(100000, 3200000)

# ------------------------------------------------------- layout planning

def plan_layout(cfg, R_wj, M_wj):
    """From per-(window, slice) padded row counts (multiples of 128), build
    the chunk-major layout shared by host preprocessing and the device
    program.

    Returns dict with:
      chunks: list of dicts with w0, nw, g0 (global subtile base),
              gch (subtiles in chunk), idx0 (global idx col base),
              regions: per (j, w): (msg subtile base within chunk, n_sub)
              slices: per j: list of (msg subtile base within chunk,
                                      idx col base within chunk, n_idx)
      NSUB, TOTROWS, IDXCOLS, GCH_MAX, KW_MAX, ICH_MAX
    """
    TN, CW = cfg.TN, cfg.CW
    chunks = []
    g0 = 0
    idx0 = 0
    KW_MAX = 0
    for w0 in range(0, TN, CW):
        nw = min(CW, TN - w0)
        regions = {}
        s = 0
        for j in range(4):
            for w in range(w0, w0 + nw):
                ns = R_wj[w][j] // 128
                regions[(j, w)] = (s, ns)
                s += ns
        gch = s
        # gather slices: per (j, w) so padding rows sit at each slice tail
        # and a runtime count register can skip them entirely
        slices = []
        icols = 0
        for j in range(4):
            jsl = []
            for w in range(w0, w0 + nw):
                base, ns = regions[(j, w)]
                rows = M_wj[w][j]          # only the max-real prefix is gathered
                r0 = 0
                while r0 < rows:
                    n = min(cfg.GMAX, rows - r0)
                    jsl.append((base + r0 // 128, icols, n, w, r0))
                    icols += (n + 15) // 16
                    r0 += n
            slices.append(jsl)
        for w in range(w0, w0 + nw):
            KW_MAX = max(KW_MAX, sum(R_wj[w][j] for j in range(4)) // 128)
        chunks.append(dict(w0=w0, nw=nw, g0=g0, gch=gch, idx0=idx0,
                           regions=regions, slices=slices, icols=icols))
        g0 += gch
        idx0 += icols
    nslices = sum(len(jsl) for c in chunks for jsl in c["slices"])
    return dict(chunks=chunks, NSUB=g0, TOTROWS=g0 * 128, IDXCOLS=idx0,
                GCH_MAX=max(c["gch"] for c in chunks),
                ICH_MAX=max(c["icols"] for c in chunks),
                KW_MAX=KW_MAX, NSLICES=nslices)

# ------------------------------------------------------- host preprocessing

def preprocess(cfg, edge_index):
    """Vectorized host prep: per-core gather index + segrel arrays.

    Returns dict with R_wj (layout key), layout, and per-core arrays:
      idx   [C, 128, IDXCOLS] int16   (wrapped per gather slice, 8x replicated)
      segrel[C, 128, NSUB]    bf16    (dst-rel per row, -1 for pad rows)
      dinv_nm [C, 128, TN]    f32
    """
    N, C, NL, SL, TN, CW = cfg.N, cfg.C, cfg.NL, cfg.SL, cfg.TN, cfg.CW
    WN = cfg.WN
    src = np.asarray(edge_index[0], dtype=np.int64)
    dst = np.asarray(edge_index[1], dtype=np.int64)
    deg = np.bincount(dst, minlength=N).astype(np.float64) + 1.0
    dinv = (1.0 / np.sqrt(deg)).astype(np.float32)

    c = dst // NL
    wrel = (dst % NL) // WN
    drel = (dst % NL) % WN
    lsrc = src % NL
    j = lsrc // SL
    tj_row = (src // NL) * SL + (lsrc - j * SL)      # row in T_j

    # padded row counts per (w, j): max over cores, rounded to 128
    key_cwj = (c * TN + wrel) * 4 + j
    cnt = np.bincount(key_cwj, minlength=C * TN * 4).reshape(C, TN, 4)
    R_wj = ((cnt.max(axis=0) + 127) // 128 * 128).astype(np.int64)  # [TN, 4]
    R_wj = np.maximum(R_wj, 128)
    M_wj = ((cnt.max(axis=0) + 15) // 16 * 16).astype(np.int64)     # [TN, 4]
    M_wj = np.maximum(M_wj, 16)

    layout = plan_layout(cfg, R_wj.tolist(), M_wj.tolist())
    NSUB, TOTROWS, IDXCOLS = layout["NSUB"], layout["TOTROWS"], layout["IDXCOLS"]

    # global row base for each (c?, w, j) region: same for all cores
    row_base = np.zeros((TN, 4), dtype=np.int64)
    for ch in layout["chunks"]:
        for (jj, w), (s, ns) in ch["regions"].items():
            row_base[w, jj] = (ch["g0"] + s) * 128

    # assign positions: sort edges by (c, w, j), rank within group
    order = np.lexsort((src, j, wrel, c))
    kc, kw, kj = c[order], wrel[order], j[order]
    key_s = (kc * TN + kw) * 4 + kj
    grp_start = np.zeros(len(key_s), dtype=np.int64)
    newgrp = np.empty(len(key_s), dtype=bool)
    newgrp[0] = True
    newgrp[1:] = key_s[1:] != key_s[:-1]
    starts = np.nonzero(newgrp)[0]
    grp_of = np.cumsum(newgrp) - 1
    rank = np.arange(len(key_s)) - starts[grp_of]
    pos = row_base[kw, kj] + rank                    # [E] row position

    gidx_rows = np.zeros((C, TOTROWS), dtype=np.int16)   # pads gather row 0
    segrel_rows = np.full((C, TOTROWS), -1.0, dtype=np.float32)
    gidx_rows[kc, pos] = tj_row[order].astype(np.int16)
    segrel_rows[kc, pos] = drel[order]

    # idx wrap: per gather slice of n rows, block[p, col] = rows[col*16 + p];
    # also per-core real-index counts per slice (runtime num_idxs_reg)
    idx = np.zeros((C, 16, IDXCOLS), dtype=np.int16)
    for ch in layout["chunks"]:
        for jj, jsl in enumerate(ch["slices"]):
            for (msub, icol, n, w, r0w) in jsl:
                r0 = (ch["g0"] + msub) * 128
                blk = gidx_rows[:, r0:r0 + n]               # [C, n]
                icc = ch["idx0"] + icol
                idx[:, :, icc:icc + n // 16] = \
                    blk.reshape(C, n // 16, 16).transpose(0, 2, 1)
    idx = np.tile(idx, (1, 8, 1))                           # [C, 128, IDXCOLS]

    segrel = segrel_rows.reshape(C, NSUB, 128).transpose(0, 2, 1).astype(BF16)

    dinv_nm = np.zeros((C, 128, TN), dtype=np.float32)
    for cc in range(C):
        w_of = np.arange(cfg.NLP) // 128
        p_of = np.arange(cfg.NLP) % 128
        v = cc * NL + w_of * WN + p_of
        valid = (p_of < WN) & (v < (cc + 1) * NL)
        dd = np.where(valid, dinv[np.minimum(v, N - 1)], 0.0)
        dinv_nm[cc] = dd.reshape(TN, 128).T

    return dict(R_wj=R_wj, M_wj=M_wj, layout=layout, idx=idx, segrel=segrel,
                dinv_nm=dinv_nm)

# ------------------------------------------------------------ device program

def build_program(cfg, R_wj, M_wj=None, debug=False):
    from concourse import bass, bacc, mybir, tile
    from concourse.masks import make_identity
    import contextlib

    f32 = mybir.dt.float32
    bf16 = mybir.dt.bfloat16
    i16 = mybir.dt.int16

    N, NL, SL, TJ, NLP, TN, D_IN, DH, DC = (
        cfg.N, cfg.NL, cfg.SL, cfg.TJ, cfg.NLP, cfg.TN, cfg.D_IN, cfg.DH,
        cfg.DC)
    if M_wj is None:
        M_wj = R_wj
    layout = plan_layout(cfg, R_wj, M_wj)
    NSUB, IDXCOLS = layout["NSUB"], layout["IDXCOLS"]
    GCH_MAX, ICH_MAX, KW_MAX = (layout["GCH_MAX"], layout["ICH_MAX"],
                                layout["KW_MAX"])
    KC = D_IN // 128
    TW = 128                                 # table row width (256B bf16)
    rg = [list(range(cfg.C))]

    nc = bacc.Bacc("TRN2", target_bir_lowering=False, debug=False,
                   num_devices=cfg.C)

    # -------- kernel I/O
    x_bf = nc.dram_tensor("x_bf", [NLP, D_IN], bf16, kind="ExternalInput").ap()
    idx_d = nc.dram_tensor("gidx", [128, IDXCOLS], i16, kind="ExternalInput").ap()
    segrel_d = nc.dram_tensor("segrel", [128, NSUB], bf16, kind="ExternalInput").ap()
    dinv_d = nc.dram_tensor("dinv_nm", [128, TN], f32, kind="ExternalInput").ap()
    w0_d = nc.dram_tensor("w0", [D_IN, DH], bf16, kind="ExternalInput").ap()
    w1_d = nc.dram_tensor("w1", [DH, DH], f32, kind="ExternalInput").ap()
    wf_d = nc.dram_tensor("wf", [DH, DC], f32, kind="ExternalInput").ap()
    b0_d = nc.dram_tensor("b0", [128, DH], f32, kind="ExternalInput").ap()
    b1_d = nc.dram_tensor("b1", [128, DH], f32, kind="ExternalInput").ap()
    bf_d = nc.dram_tensor("bf_", [128, DC], f32, kind="ExternalInput").ap()
    iota_d = nc.dram_tensor("iota_row", [128, 128], bf16, kind="ExternalInput").ap()
    out_d = nc.dram_tensor("out", [NL, DC], f32, kind="ExternalOutput").ap()

    # -------- internal DRAM
    tq = [nc.dram_tensor(f"tq{j}", [TJ, TW], bf16).ap() for j in range(4)]
    shard = nc.dram_tensor("h_shard", [NL, TW], bf16).ap()
    st_in = nc.dram_tensor("st_in", [1, 64], f32).ap()
    st_out = nc.dram_tensor("st_out", [1, 64], f32).ap()

    names = ["x_bf", "gidx", "segrel", "dinv_nm", "w0", "w1", "wf",
             "b0", "b1", "bf_", "iota_row"]

    with tile.TileContext(nc) as tc:
        with contextlib.ExitStack() as ctx:
            big = ctx.enter_context(tc.tile_pool(name="big", bufs=1))

            # ---- residents
            segrel_sb = big.tile([128, NSUB], bf16)
            dinv_sb = big.tile([128, TN], f32)
            w0_sb = big.tile([128, KC, DH], bf16)
            w1_sb = big.tile([DH, DH], f32)
            wf_sb = big.tile([DH, DC], f32)
            b0_sb = big.tile([128, DH], f32)
            b1_sb = big.tile([128, DH], f32)
            bf_sb = big.tile([128, DC], f32)
            iota_sb = big.tile([128, 128], bf16)
            ident = big.tile([128, 128], f32)
            ones_sb = big.tile([128, 1], f32)
            ones_row = big.tile([1, 128], f32)
            stats_sb = big.tile([1, 64], f32)

            hloc = big.tile([128, TN, DH], f32)
            agg = big.tile([128, TN, DH], f32)
            state = big.tile([128, TN, DH], f32)
            shard_sb = big.tile([128, TN, TW], bf16)
            final_sb = big.tile([128, TN, DC], f32)
            sq = final_sb[:, :, 0:DH]

            nc.sync.dma_start(out=segrel_sb[:], in_=segrel_d[:])
            nc.sync.dma_start(out=dinv_sb[:], in_=dinv_d[:])
            nc.sync.dma_start(out=w0_sb[:], in_=w0_d.rearrange("(c p) f -> p c f", p=128))
            nc.sync.dma_start(out=w1_sb[:], in_=w1_d[:])
            nc.sync.dma_start(out=wf_sb[:], in_=wf_d[:])
            nc.sync.dma_start(out=b0_sb[:], in_=b0_d[:])
            nc.sync.dma_start(out=b1_sb[:], in_=b1_d[:])
            nc.sync.dma_start(out=bf_sb[:], in_=bf_d[:])
            nc.sync.dma_start(out=iota_sb[:], in_=iota_d[:])
            make_identity(nc, ident[:])
            nc.vector.memset(ones_sb[:], 1.0)
            nc.vector.memset(ones_row[:], 1.0)
            nc.vector.memset(shard_sb[:], 0.0)   # cols DH..TW stay zero

            def dinv_b(ap_shape):
                return dinv_sb[:, :, None].to_broadcast(ap_shape)

            # ---------------- phase A: t0 = x @ W0 (bf16), hloc = dinv*t0
            TH0 = (TN + 1) // 2
            with tc.tile_pool(name="psA", bufs=2, space="PSUM") as psA:
                with tc.tile_pool(name="xTp", bufs=1) as xTp:
                    for h, (tlo, thi) in enumerate([(0, TH0), (TH0, TN)]):
                        nh = thi - tlo
                        if nh <= 0:
                            continue
                        xT = [xTp.tile([128, TH0 * 128], bf16, tag=f"xT{kk}",
                                       name=f"xT{kk}_{h}") for kk in range(KC)]
                        for kk in range(KC):
                            nc.sync.dma_start_transpose(
                                out=xT[kk][:, 0:nh * 128],
                                in_=x_bf[tlo * 128: thi * 128,
                                         128 * kk:128 * (kk + 1)])
                        for t in range(tlo, thi):
                            t0 = psA.tile([128, DH], f32, tag="psA",
                                          name=f"t0_{t}")
                            for kk in range(KC):
                                nc.tensor.matmul(
                                    out=t0[:],
                                    lhsT=xT[kk][:, 128 * (t - tlo):128 * (t - tlo + 1)],
                                    rhs=w0_sb[:, kk, :],
                                    start=(kk == 0), stop=(kk == KC - 1))
                            nc.vector.tensor_tensor(
                                out=hloc[:, t, :], in0=t0[:],
                                in1=dinv_sb[:, t:t + 1].to_broadcast([128, DH]),
                                op=mybir.AluOpType.mult)

            WN = cfg.WN
            NFW = NL // WN                     # full windows per core

            def write_table_and_allgather():
                nc.vector.tensor_copy(out=shard_sb[:, :, 0:DH], in_=hloc[:])
                nc.sync.dma_start(
                    out=shard[0:NFW * WN, :].rearrange("(t p) f -> p t f", p=WN),
                    in_=shard_sb[0:WN, 0:NFW, :])
                if NL > NFW * WN:
                    nc.sync.dma_start(
                        out=shard[NFW * WN:NL, :],
                        in_=shard_sb[0:NL - NFW * WN, NFW, :])
                for j in range(4):
                    nc.gpsimd.collective_compute(
                        "AllGather", mybir.AluOpType.bypass,
                        replica_groups=rg,
                        ins=[shard[j * SL:(j + 1) * SL, :]],
                        outs=[tq[j][:]],
                    )

            write_table_and_allgather()

            # ---------------- layers
            msgp = ctx.enter_context(tc.tile_pool(name="msg", bufs=2))
            ohp = ctx.enter_context(tc.tile_pool(name="oh", bufs=2))
            idxp = ctx.enter_context(tc.tile_pool(name="idx", bufs=2))
            smp = ctx.enter_context(tc.tile_pool(name="sm", bufs=4))
            psw = ctx.enter_context(tc.tile_pool(name="psw", bufs=2, space="PSUM"))
            pstr = ctx.enter_context(tc.tile_pool(name="pstr", bufs=2, space="PSUM"))
            psst = ctx.enter_context(tc.tile_pool(name="psst", bufs=1, space="PSUM"))

            for layer in range(3):
                # ---- gather + one-hot segment-sum, chunk by chunk
                for ci, ch in enumerate(layout["chunks"]):
                    msg = msgp.tile([128, GCH_MAX, TW], bf16, tag="msg")
                    idx_sb = idxp.tile([128, ICH_MAX], i16, tag="idx")
                    # rows skipped by the runtime count keep stale SBUF
                    # contents; zero the matmul-visible columns so
                    # 0-one-hot x garbage can never make NaN
                    nc.vector.memset(msg[:, 0:ch["gch"], 0:DH], 0.0)
                    nc.sync.dma_start(
                        out=idx_sb[:, 0:ch["icols"]],
                        in_=idx_d[:, ch["idx0"]:ch["idx0"] + ch["icols"]])
                    for j in range(4):
                        for (msub, icol, n, _w, _r0) in ch["slices"][j]:
                            nc.gpsimd.dma_gather(
                                out_ap=msg[:, msub:msub + (n + 127) // 128, :],
                                in_ap=tq[j][:],
                                idxs_ap=idx_sb[:, icol:icol + (n + 15) // 16],
                                num_idxs=n, num_idxs_reg=n, elem_size=TW)
                    for w in range(ch["w0"], ch["w0"] + ch["nw"]):
                        kws = [ch["regions"][(j, w)] for j in range(4)]
                        kw_w = sum(ns for (_, ns) in kws)
                        oh = ohp.tile([128, KW_MAX, 128], bf16, tag="oh")
                        k = 0
                        for (s, ns) in kws:
                            nc.vector.tensor_tensor(
                                out=oh[:, k:k + ns, :],
                                in0=iota_sb[:, None, :].to_broadcast([128, ns, 128]),
                                in1=segrel_sb[:, ch["g0"] + s: ch["g0"] + s + ns,
                                              None].to_broadcast([128, ns, 128]),
                                op=mybir.AluOpType.is_equal)
                            k += ns
                        pw = psw.tile([128, DH], f32, tag="win")
                        k = 0
                        for (s, ns) in kws:
                            for si in range(ns):
                                nc.tensor.matmul(
                                    out=pw[:],
                                    lhsT=oh[:, k, :],
                                    rhs=msg[:, s + si, 0:DH],
                                    start=(k == 0), stop=(k == kw_w - 1))
                                k += 1
                        # agg_w = segsum + hloc (fused psum evacuation)
                        nc.vector.tensor_tensor(
                            out=agg[:, w, :], in0=pw[:], in1=hloc[:, w, :],
                            op=mybir.AluOpType.add)

                # ---- pre = dinv * (agg + hloc-already-added)
                nc.vector.tensor_tensor(out=agg[:], in0=agg[:],
                                        in1=dinv_b([128, TN, DH]),
                                        op=mybir.AluOpType.mult)

                # ---- out_k = pre @ W + b  (layer 0: W already applied)
                if layer == 0:
                    nc.vector.tensor_tensor(
                        out=state[:], in0=agg[:],
                        in1=b0_sb[:, None, :].to_broadcast([128, TN, DH]),
                        op=mybir.AluOpType.add)
                else:
                    W_sb, b_sb, DO = ((w1_sb, b1_sb, DH) if layer == 1
                                      else (wf_sb, bf_sb, DC))
                    dst_buf = state if layer == 1 else final_sb
                    for t in range(TN):
                        ptp = pstr.tile([DH, 128], f32, tag="small", name=f"tr{t}")
                        nc.tensor.transpose(out=ptp[:], in_=agg[:, t, :],
                                            identity=ident[:])
                        preT = smp.tile([DH, 128], f32, tag="preT")
                        nc.scalar.activation(out=preT[:], in_=ptp[:],
                                             func=mybir.ActivationFunctionType.Copy)
                        po = pstr.tile([128, DO], f32, tag="small", name=f"po{t}")
                        nc.tensor.matmul(out=po[:], lhsT=preT[:], rhs=W_sb[:, 0:DO],
                                         start=True, stop=True)
                        nc.vector.tensor_tensor(
                            out=dst_buf[:, t, 0:DO], in0=po[:],
                            in1=b_sb[:, 0:DO],
                            op=mybir.AluOpType.add)

                if layer == 2:
                    nc.sync.dma_start(
                        out=out_d[0:(TN - 1) * WN, :].rearrange(
                            "(t p) f -> p t f", p=WN),
                        in_=final_sb[0:WN, 0:TN - 1, :])
                    lastn = NL - (TN - 1) * WN
                    nc.sync.dma_start(
                        out=out_d[(TN - 1) * WN: NL, :],
                        in_=final_sb[0:lastn, TN - 1, :])
                    continue

                # ---- pairnorm stats: colsums of state and state^2
                nc.vector.tensor_tensor(out=sq, in0=state[:], in1=state[:],
                                        op=mybir.AluOpType.mult)
                ps_s = psst.tile([DH, 1], f32, tag="st_s")
                ps_q = psst.tile([DH, 1], f32, tag="st_q")
                for t in range(TN):
                    kk = WN if t < TN - 1 else (NL - (TN - 1) * WN)
                    nc.tensor.matmul(out=ps_s[:], lhsT=state[0:kk, t, :],
                                     rhs=ones_sb[0:kk, :],
                                     start=(t == 0), stop=(t == TN - 1))
                    nc.tensor.matmul(out=ps_q[:], lhsT=sq[0:kk, t, :],
                                     rhs=ones_sb[0:kk, :],
                                     start=(t == 0), stop=(t == TN - 1))
                s_sb = smp.tile([DH, 2], f32, tag="s_sb")
                nc.vector.tensor_copy(out=s_sb[:, 0:1], in_=ps_s[:])
                nc.vector.tensor_copy(out=s_sb[:, 1:2], in_=ps_q[:])
                nc.sync.dma_start(out=st_in.rearrange("o (p f) -> (o p) f", f=2),
                                  in_=s_sb[:])
                nc.gpsimd.collective_compute(
                    "AllReduce", mybir.AluOpType.add, replica_groups=rg,
                    ins=[st_in[:]], outs=[st_out[:]])
                nc.sync.dma_start(out=stats_sb[:], in_=st_out[:])
                s_ap = stats_sb[:].rearrange("o (p f) -> o p f", f=2)
                mean = smp.tile([1, DH], f32, tag="mean")
                nc.vector.tensor_scalar(
                    out=mean[:], in0=s_ap[:, :, 0], scalar1=1.0 / cfg.N,
                    scalar2=None, op0=mybir.AluOpType.mult)
                m2 = smp.tile([1, DH], f32, tag="m2")
                nc.vector.tensor_tensor(out=m2[:], in0=mean[:],
                                        in1=s_ap[:, :, 0],
                                        op=mybir.AluOpType.mult)
                r1 = smp.tile([1, 1], f32, tag="r1")
                nc.vector.reduce_sum(out=r1[:], in_=m2[:],
                                     axis=mybir.AxisListType.X)
                qs = smp.tile([1, 1], f32, tag="qs")
                nc.vector.reduce_sum(out=qs[:], in_=s_ap[:, :, 1],
                                     axis=mybir.AxisListType.X)
                v_ = smp.tile([1, 1], f32, tag="v_")
                nc.vector.tensor_tensor(out=v_[:], in0=qs[:], in1=r1[:],
                                        op=mybir.AluOpType.subtract)
                nc.vector.tensor_scalar(
                    out=v_[:], in0=v_[:], scalar1=1.0 / cfg.N,
                    scalar2=cfg.EPS, op0=mybir.AluOpType.mult,
                    op1=mybir.AluOpType.add)
                rt = smp.tile([1, 1], f32, tag="rt")
                nc.scalar.activation(out=rt[:], in_=v_[:],
                                     func=mybir.ActivationFunctionType.Sqrt)
                scl = smp.tile([1, 1], f32, tag="scl")
                nc.vector.reciprocal(out=scl[:], in_=rt[:])

                msc = smp.tile([1, DH + 1], f32, tag="msc")
                nc.vector.tensor_copy(out=msc[:, 0:DH], in_=mean[:])
                nc.vector.tensor_copy(out=msc[:, DH:DH + 1], in_=scl[:])
                pmsc = pstr.tile([128, DH + 1], f32, tag="small",
                                 name=f"pmsc{layer}")
                nc.tensor.matmul(out=pmsc[:], lhsT=ones_row[0:1, :],
                                 rhs=msc[:], start=True, stop=True)
                msc128 = smp.tile([128, DH + 1], f32, tag="msc128")
                nc.vector.tensor_copy(out=msc128[:], in_=pmsc[:])

                # ---- z = relu((state - mean) * scale); hloc = dinv * z
                nc.vector.tensor_tensor(
                    out=state[:], in0=state[:],
                    in1=msc128[:, None, 0:DH].to_broadcast([128, TN, DH]),
                    op=mybir.AluOpType.subtract)
                nc.vector.tensor_tensor(
                    out=state[:], in0=state[:],
                    in1=msc128[:, None, DH:DH + 1].to_broadcast([128, TN, DH]),
                    op=mybir.AluOpType.mult)
                nc.vector.tensor_scalar(
                    out=state[:], in0=state[:], scalar1=0.0, scalar2=None,
                    op0=mybir.AluOpType.max)
                nc.vector.tensor_tensor(out=hloc[:], in0=state[:],
                                        in1=dinv_b([128, TN, DH]),
                                        op=mybir.AluOpType.mult)
                write_table_and_allgather()

    nc.compile()
    return nc, names


# ----------------------------------------------------------------- entry

def make_inputs(cfg, pp, x, W0, b0, W1, b1, Wf, bf):
    C, NL, NLP, WN, TN = cfg.C, cfg.NL, cfg.NLP, cfg.WN, cfg.TN
    x_pad = np.zeros((C, NLP, cfg.D_IN), dtype=BF16)
    for c in range(C):
        xw = np.zeros((TN * WN, cfg.D_IN), dtype=BF16)
        xw[:NL] = x[c * NL:(c + 1) * NL].astype(BF16)
        x_pad[c].reshape(TN, 128, cfg.D_IN)[:, :WN, :] = \
            xw.reshape(TN, WN, cfg.D_IN)
    iota_row = np.tile(np.arange(128, dtype=np.float32).astype(BF16)[None, :],
                       (128, 1))
    in_maps = []
    for c in range(C):
        in_maps.append({
            "x_bf": x_pad[c],
            "gidx": pp["idx"][c],
            "segrel": pp["segrel"][c],
            "dinv_nm": pp["dinv_nm"][c],
            "w0": W0.astype(BF16),
            "w1": W1.astype(np.float32),
            "wf": Wf.astype(np.float32),
            "b0": np.tile(b0.reshape(1, -1).astype(np.float32), (128, 1)),
            "b1": np.tile(b1.reshape(1, -1).astype(np.float32), (128, 1)),
            "bf_": np.tile(bf.reshape(1, -1).astype(np.float32), (128, 1)),
            "iota_row": iota_row,
        })
    return in_maps


_CACHE = {}

def kernel(x, edge_index, W0, b0, W1, b1, Wf, bf):
    from concourse import bass_utils
    cfg = FULL
    x = np.asarray(x)
    edge_index = np.asarray(edge_index)
    pp = preprocess(cfg, edge_index)
    key = ("prog", pp["R_wj"].tobytes(), pp["M_wj"].tobytes())
    if key not in _CACHE:
        _CACHE[key] = build_program(cfg, pp["R_wj"].tolist(),
                                    pp["M_wj"].tolist())
    nc, _names = _CACHE[key]
    in_maps = make_inputs(cfg, pp, x, W0, b0, W1, b1, Wf, bf)
    # the axon-tunneled devices occasionally come up wedged from a prior
    # process (NRT_EXEC_UNIT_UNRECOVERABLE); a retry has always recovered
    for attempt in range(3):
        try:
            res = bass_utils.run_bass_kernel_spmd(nc, in_maps, list(range(cfg.C)))
            break
        except Exception:                                 # noqa: BLE001
            if attempt == 2:
                raise
            import time
            time.sleep(2.0)
    out = np.concatenate([res.results[c]["out"] for c in range(cfg.C)], axis=0)
    return out.astype(np.float32)
